# revision 1
# baseline (speedup 1.0000x reference)
"""Trainium2 Bass kernel for a 2-layer GCN (AblationGCN) on 8 NeuronCores.

Contract: kernel(**inputs) takes the FULL unsharded inputs of the reference
(x [100000,165] f32, edge_index [2,1600000] int, W1,b1,W2,b2,Wc,bc) and
returns the FULL output [100000, 2] f32.

Strategy (SPMD, one Bass program on cores 0-7):
  - dst-range sharding with a per-core node permutation that balances
    per-128-node-block edge counts. Global table position of node v is
    gpos(v) = core(v)*12544 + perm(v).
  - L1 dense SHARDED: each core computes g1 = dinv*(x_shard @ W1) for its
    own 12544 positions, then one AllGather builds the full node-major
    bf16 table g_t [100352, 128] on every core.
  - Aggregation (both layers): edges+self-loops are grouped by
    (src-chunk, dst-block) where chunks are 32768-row ranges of the table
    (int16 index reach of gpsimd.dma_gather). Big batched dma_gather
    instructions (up to 4096 edges each) stream gathered rows into SBUF;
    per (chunk,block) cell a one-hot matmul S^T @ G accumulates in PSUM,
    and cells are reduced across chunks into an SBUF f32 accumulator.
    Padding slots use dummy index 0 with a -1 one-hot key (zero column).
  - Per-block epilogue: *dinv[dst], +bias, relu; PE-transpose feeds both
    the L2 dense matmul (sharded, straight from SBUF) and the classifier.
  - L2 dense -> second AllGather -> L2 aggregation -> classifier -> out.
Host preprocessing computes degrees/dinv, the permutation, the uniform
per-(block,chunk) tile counts, wrapped int16 gather indices and one-hot
keys, all packed into two bf16 input blobs.
"""
import numpy as np

P = 128
N = 100000
N_CORES = 8
IN_F = 165
HID = 128
SHARD = N // N_CORES            # 12500
NB = (SHARD + P - 1) // P       # 98
SHARD_PAD = NB * P              # 12544
NPAD2 = N_CORES * SHARD_PAD     # 100352
CHUNK = 32768
NCH = (NPAD2 + CHUNK - 1) // CHUNK          # 4
RC = [min(CHUNK, NPAD2 - c * CHUNK) for c in range(NCH)]
GT = 8                          # tiles per dma_gather (1024-desc ucode ring cap)
SBATCH = 16                     # S one-hot tiles built per vector op


def _split_excess_waits(nc, max_waits=1):
    """This walrus build only accepts one sync-wait command per instruction;
    hoist extras onto NoOps placed just before the carrying instruction."""
    import concourse.mybir as mybir
    for f in nc.m.functions:
        for b in f.blocks:
            insts = b.instructions
            new_list = []
            changed = False
            for ins in insts:
                si = ins.sync_info
                if si is not None and len(si.on_wait) > max_waits:
                    waits = list(si.on_wait)
                    keep = waits[:max_waits]
                    extra = waits[max_waits:]
                    for ci in range(0, len(extra), max_waits):
                        chunk = extra[ci:ci + max_waits]
                        nop = mybir.InstNoOp(name=f"{ins.name}_wsplit{ci}",
                                             ins=[], outs=[])
                        nop.engine = ins.engine
                        nop.sync_info = mybir.SyncInfo(on_wait=chunk, on_update=[])
                        new_list.append(nop)
                    si.on_wait = keep
                    changed = True
                new_list.append(ins)
            if changed:
                b.instructions = new_list


def _build(tbc, reps=1, no_coll=False):
    """tbc: [NB][NCH] compile-time tiles per (block, chunk) cell.
    reps>1 repeats the whole (idempotent) pipeline for slope timing."""
    import concourse.bass as bass
    import concourse.mybir as mybir
    import concourse.tile as tile
    from concourse.library_config import mlp
    F32 = mybir.dt.float32
    BF16 = mybir.dt.bfloat16
    I16 = mybir.dt.int16
    AF = mybir.ActivationFunctionType
    T = int(sum(sum(r) for r in tbc))
    C16 = T * 8
    import os
    no_coll = no_coll or bool(os.environ.get("K_NO_COLL"))
    nc = bass.Bass()

    # Packed input blobs (per-call dispatch overhead scales with param count).
    WF = HID + HID + 2 + NB                       # b1|b2|bc|dinv_blk (f32)
    W16 = SHARD_PAD + HID + HID + 2 + P + P + T + 2 * WF + C16
    W16 += W16 % 2                                # even width for f32 bitcast
    W37 = SHARD_PAD + HID                         # xt_b|w1b
    cb16 = nc.declare_dram_parameter("cb16", [P, W16], BF16, isOutput=False)
    cb37 = nc.declare_dram_parameter("cb37", [IN_F - P, W37], BF16,
                                     isOutput=False)
    out = nc.declare_dram_parameter("out", [SHARD, 2], F32, isOutput=True)

    o_w1a = SHARD_PAD
    o_w2 = o_w1a + HID
    o_wc = o_w2 + HID
    o_iota = o_wc + 2
    o_id = o_iota + P
    o_f32 = o_id + P
    o_dst = o_f32 + 2 * WF
    o_i16 = o_dst + T
    assert o_f32 % 2 == 0

    def _f32(off, w):
        return cb16[:, o_f32 + 2 * off:o_f32 + 2 * (off + w)].bitcast(F32)

    of_b1, of_b2, of_bc, of_dbk = 0, HID, 2 * HID, 2 * HID + 2

    # chunk start tile offsets in the stream: order is (c, b, t)
    ch_tiles = [int(sum(tbc[b][c] for b in range(NB))) for c in range(NCH)]
    ch_start = [0] * NCH
    for c in range(1, NCH):
        ch_start[c] = ch_start[c - 1] + ch_tiles[c - 1]

    with tile.TileContext(nc) as tc:
        nc.gpsimd.load_library(mlp)
        with tc.tile_pool(name="const", bufs=1) as constp, \
             tc.tile_pool(name="dram", bufs=1, space="DRAM") as dramp:

            def load_const(name, ap, shape, dt):
                t = constp.tile(shape, dt, name=name)
                nc.sync.dma_start(out=t[:], in_=ap)
                return t

            w1a_s = load_const("w1a_s", cb16[:, o_w1a:o_w1a + HID],
                               [P, HID], BF16)
            w1b_s = load_const("w1b_s", cb37[:, SHARD_PAD:SHARD_PAD + HID],
                               [IN_F - P, HID], BF16)
            w2_s = load_const("w2_s", cb16[:, o_w2:o_w2 + HID], [HID, HID],
                              BF16)
            wc_s = load_const("wc_s", cb16[:, o_wc:o_wc + 2], [HID, 2], BF16)
            iota_s = load_const("iota_s", cb16[:, o_iota:o_iota + P],
                                [P, P], BF16)
            id_s = load_const("id_s", cb16[:, o_id:o_id + P], [P, P], BF16)
            b1_s = load_const("b1_s", _f32(of_b1, HID), [P, HID], F32)
            b2_s = load_const("b2_s", _f32(of_b2, HID), [P, HID], F32)
            bc_s = load_const("bc_s", _f32(of_bc, 2), [P, 2], F32)
            dinv_blk_s = load_const("dinv_blk_s", _f32(of_dbk, NB),
                                    [P, NB], F32)
            dstloc_s = load_const("dstloc_s", cb16[:, o_dst:o_dst + T],
                                  [P, T], BF16)
            idx_s = load_const("idx_s", cb16[:, o_i16:o_i16 + C16].bitcast(I16),
                               [P, C16], I16)

            g_loc = dramp.tile([SHARD_PAD, HID], BF16, name="g_loc")

            for rep in range(reps):
              g_t1 = dramp.tile([NPAD2, HID], BF16, addr_space="Shared",
                                name=f"g_t1_{rep}")
              g_t2 = dramp.tile([NPAD2, HID], BF16, addr_space="Shared",
                                name=f"g_t2_{rep}")

              # ---------------- L1 dense (sharded) ----------------
              with tc.tile_pool(name=f"xslab{rep}", bufs=1) as xsp, \
                 tc.tile_pool(name=f"gstage{rep}", bufs=2) as gsp, \
                 tc.tile_pool(name=f"psD{rep}", bufs=4, space="PSUM") as psD:
                xa = xsp.tile([P, SHARD_PAD], BF16, name="xa")
                nc.sync.dma_start(out=xa[:], in_=cb16[:, :SHARD_PAD])
                xb = xsp.tile([IN_F - P, SHARD_PAD], BF16, name="xb")
                nc.sync.dma_start(out=xb[:], in_=cb37[:, :SHARD_PAD])
                gst = None
                for b in range(NB):
                    if b % 16 == 0:
                        gst = gsp.tile([P, 16 * HID], BF16, tag="gst",
                                       name="gst")
                    ps = psD.tile([P, HID], F32, tag="psD", name="psd")
                    nc.tensor.matmul(ps[:], lhsT=xa[:, b * P:(b + 1) * P],
                                     rhs=w1a_s[:], start=True, stop=False)
                    nc.tensor.matmul(ps[:], lhsT=xb[:, b * P:(b + 1) * P],
                                     rhs=w1b_s[:], start=False, stop=True)
                    nc.scalar.activation(
                        gst[:, (b % 16) * HID:(b % 16 + 1) * HID], ps[:],
                        AF.Copy, scale=dinv_blk_s[:, b:b + 1])
                    if b % 16 == 15 or b == NB - 1:
                        b0 = (b // 16) * 16
                        nb_ = b - b0 + 1
                        nc.sync.dma_start(
                            out=g_loc[b0 * P:(b + 1) * P, :].rearrange(
                                "(t p) f -> p t f", p=P),
                            in_=gst[:, :nb_ * HID].rearrange(
                                "p (t f) -> p t f", f=HID))

              if not no_coll:
                nc.gpsimd.collective_compute(
                    "AllGather", mybir.AluOpType.bypass,
                    replica_groups=[list(range(N_CORES))],
                    ins=[g_loc[:]], outs=[g_t1[:]])
              else:
                nc.sync.dma_start(out=g_t1[:SHARD_PAD, :], in_=g_loc[:])

              # shared SBUF f32 accumulator [dst-in-block, block*feat]
              with tc.tile_pool(name=f"accp{rep}", bufs=1) as accp:
                acc = accp.tile([P, NB * HID], F32, name="acc")

                nreg = {}

                def _nreg(R):
                    if R not in nreg:
                        nreg[R] = nc.gpsimd.to_reg(R)
                    return nreg[R]

                def aggregation(g_t, bias_s, out_blk_cb, phase):
                    with tc.tile_pool(name=f"gt{phase}{rep}", bufs=3) as gtp, \
                         tc.tile_pool(name=f"sp{phase}{rep}", bufs=3) as spp, \
                         tc.tile_pool(name=f"ps{phase}{rep}", bufs=4,
                                      space="PSUM") as psp:
                        S = None
                        G = None
                        gi = 0
                        for c in range(NCH):
                            c0 = c * CHUNK
                            for b in range(NB):
                                nt = tbc[b][c]
                                if nt == 0:
                                    continue
                                ps = psp.tile([P, HID], F32, tag="ps",
                                              name="ps")
                                for t in range(nt):
                                    pic = gi - ch_start[c]  # pos in chunk
                                    if pic % GT == 0:
                                        rt = min(GT, ch_tiles[c] - pic)
                                        R = rt * 128
                                        G = gtp.tile([P, GT * HID], BF16,
                                                     tag="G", name="G")
                                        nc.gpsimd.dma_gather(
                                            out_ap=G[:, :rt * HID].rearrange(
                                                "p (t f) -> p t f", f=HID),
                                            in_ap=g_t[c0:c0 + RC[c], :],
                                            idxs_ap=idx_s[:, gi * 8:
                                                          gi * 8 + R // 16],
                                            num_idxs=R,
                                            num_idxs_reg=_nreg(R),
                                            elem_size=HID)
                                    if gi % SBATCH == 0:
                                        nw = min(SBATCH, T - gi)
                                        S = spp.tile([P, SBATCH * P], BF16,
                                                     tag="S", name="S")
                                        iota_b = bass.AP(
                                            iota_s[:].tensor,
                                            iota_s[:].offset,
                                            [iota_s[:].ap[0], [0, nw],
                                             iota_s[:].ap[1]])
                                        dst_b = dstloc_s[:, gi:gi + nw]\
                                            .to_broadcast([P, nw, P])
                                        nc.vector.tensor_tensor(
                                            out=S[:, :nw * P].rearrange(
                                                "p (t f) -> p t f", f=P),
                                            in0=iota_b, in1=dst_b,
                                            op=mybir.AluOpType.is_equal)
                                    nc.tensor.matmul(
                                        ps[:],
                                        lhsT=S[:, (gi % SBATCH) * P:
                                               (gi % SBATCH + 1) * P],
                                        rhs=G[:, (pic % GT) * HID:
                                              (pic % GT + 1) * HID],
                                        start=(t == 0), stop=(t == nt - 1))
                                    gi += 1
                                a_sl = acc[:, b * HID:(b + 1) * HID]
                                if c == 0:
                                    nc.vector.tensor_copy(out=a_sl, in_=ps[:])
                                else:
                                    nc.vector.tensor_tensor(
                                        out=a_sl, in0=a_sl, in1=ps[:],
                                        op=mybir.AluOpType.add)
                                if c == NCH - 1:
                                    out_blk_cb(b, a_sl, bias_s)
                        # blocks whose last chunk had no tiles
                        for b in range(NB):
                            if tbc[b][NCH - 1] == 0:
                                out_blk_cb(b, acc[:, b * HID:(b + 1) * HID],
                                           bias_s)
                        assert gi == T

                def epilogue(b, a_sl, bias_s, epp):
                    """*dinv, +bias, relu -> bf16; PE-transpose -> [feat,node]
                    tile in SBUF. Returns the transposed bf16 tile."""
                    t1 = epp.tile([P, HID], F32, tag="t1", name="t1")
                    nc.scalar.activation(t1[:], a_sl, AF.Copy,
                                         scale=dinv_blk_s[:, b:b + 1])
                    t2 = epp.tile([P, HID], F32, tag="t2", name="t2")
                    nc.vector.tensor_tensor(out=t2[:], in0=t1[:], in1=bias_s[:],
                                            op=mybir.AluOpType.add)
                    a_sb = epp.tile([P, HID], BF16, tag="a_sb", name="a_sb")
                    nc.vector.tensor_scalar_max(out=a_sb[:], in0=t2[:],
                                                scalar1=0.0)
                    return a_sb

                # ---------------- L1 agg + L2 dense ----------------
                with tc.tile_pool(name=f"epA{rep}", bufs=3) as epA, \
                     tc.tile_pool(name=f"psTA{rep}", bufs=2, space="PSUM") as psTA, \
                     tc.tile_pool(name=f"g2st{rep}", bufs=2) as g2sp, \
                     tc.tile_pool(name=f"psD2{rep}", bufs=2, space="PSUM") as psD2:
                    g2st = [None]

                    def l1_out(b, a_sl, bias_s):
                        a_sb = epilogue(b, a_sl, bias_s, epA)
                        pT = psTA.tile([P, P], BF16, tag="pT", name="pT")
                        nc.tensor.transpose(pT[:], a_sb[:], id_s[:])
                        a1T = epA.tile([P, P], BF16, tag="a1T", name="a1T")
                        nc.vector.tensor_copy(out=a1T[:], in_=pT[:])
                        # L2 dense for this block, straight from SBUF
                        ps2 = psD2.tile([P, HID], F32, tag="ps2", name="ps2")
                        nc.tensor.matmul(ps2[:], lhsT=a1T[:], rhs=w2_s[:],
                                         start=True, stop=True)
                        if b % 16 == 0:
                            g2st[0] = g2sp.tile([P, 16 * HID], BF16,
                                                tag="g2st", name="g2st")
                        nc.scalar.activation(
                            g2st[0][:, (b % 16) * HID:(b % 16 + 1) * HID],
                            ps2[:], AF.Copy, scale=dinv_blk_s[:, b:b + 1])
                        if b % 16 == 15 or b == NB - 1:
                            b0 = (b // 16) * 16
                            nb_ = b - b0 + 1
                            nc.sync.dma_start(
                                out=g_loc[b0 * P:(b + 1) * P, :].rearrange(
                                    "(t p) f -> p t f", p=P),
                                in_=g2st[0][:, :nb_ * HID].rearrange(
                                    "p (t f) -> p t f", f=HID))

                    aggregation(g_t1, b1_s, l1_out, "A")

                if not no_coll:
                    nc.gpsimd.collective_compute(
                        "AllGather", mybir.AluOpType.bypass,
                        replica_groups=[list(range(N_CORES))],
                        ins=[g_loc[:]], outs=[g_t2[:]])
                else:
                    nc.sync.dma_start(out=g_t2[:SHARD_PAD, :], in_=g_loc[:])

                # ---------------- L2 agg + classifier ----------------
                with tc.tile_pool(name=f"epB{rep}", bufs=3) as epB, \
                     tc.tile_pool(name=f"psTB{rep}", bufs=2, space="PSUM") as psTB, \
                     tc.tile_pool(name=f"psC{rep}", bufs=2, space="PSUM") as psC:

                    def l2_out(b, a_sl, bias_s):
                        a_sb = epilogue(b, a_sl, bias_s, epB)
                        pT = psTB.tile([P, P], BF16, tag="pT2", name="pT2")
                        nc.tensor.transpose(pT[:], a_sb[:], id_s[:])
                        a2T = epB.tile([P, P], BF16, tag="a2T", name="a2T")
                        nc.vector.tensor_copy(out=a2T[:], in_=pT[:])
                        pc = psC.tile([P, 2], F32, tag="pC", name="pC")
                        nc.tensor.matmul(pc[:], lhsT=a2T[:], rhs=wc_s[:],
                                         start=True, stop=True)
                        ob = epB.tile([P, 2], F32, tag="ob", name="ob")
                        nc.vector.tensor_tensor(out=ob[:], in0=pc[:],
                                                in1=bc_s[:],
                                                op=mybir.AluOpType.add)
                        nrows = min(P, SHARD - b * P)
                        nc.sync.dma_start(out=out[b * P:b * P + nrows, :],
                                          in_=ob[:nrows, :])

                    aggregation(g_t2, b2_s, l2_out, "B")

    mybir.codegen_inst_isa_subclasses(nc)
    _split_excess_waits(nc)
    return nc


def _prepare(x, edge_index, W1, b1, W2, b2, Wc, bc):
    import ml_dtypes
    bf = ml_dtypes.bfloat16
    x = np.asarray(x, np.float32)
    src = np.asarray(edge_index[0], dtype=np.int64)
    dst = np.asarray(edge_index[1], dtype=np.int64)
    deg = np.bincount(dst, minlength=N).astype(np.float32) + 1.0
    dinv = 1.0 / np.sqrt(deg)
    allsrc = np.concatenate([src, np.arange(N, dtype=np.int64)])
    alldst = np.concatenate([dst, np.arange(N, dtype=np.int64)])

    # Balance per-block edge counts: permute each core's local node
    # positions (snake assignment by degree) so every 128-node dst block has
    # a near-equal edge count.
    import heapq
    caps = np.full(NB, P, np.int64)
    caps[-1] = SHARD - (NB - 1) * P
    perms = []
    for cc in range(N_CORES):
        lo = cc * SHARD
        order = np.argsort(-deg[lo:lo + SHARD], kind="stable")
        heap = [(0.0, int(b)) for b in range(NB)]
        heapq.heapify(heap)
        fill = np.zeros(NB, np.int64)
        perm = np.empty(SHARD, np.int64)
        degs = deg[lo:lo + SHARD]
        for ol in order:
            while True:
                w, b = heapq.heappop(heap)
                if fill[b] < caps[b]:
                    break
            perm[ol] = b * P + fill[b]
            fill[b] += 1
            if fill[b] < caps[b]:
                heapq.heappush(heap, (w + float(degs[ol]), b))
        perms.append(perm)

    gpos = np.empty(N, np.int64)
    for cc in range(N_CORES):
        gpos[cc * SHARD:(cc + 1) * SHARD] = cc * SHARD_PAD + perms[cc]
    sg_all = gpos[allsrc]

    # per-core cell structure
    per_core = []
    counts_all = np.zeros((N_CORES, NB, NCH), np.int64)
    for cc in range(N_CORES):
        lo, hi = cc * SHARD, (cc + 1) * SHARD
        m = (alldst >= lo) & (alldst < hi)
        sgl = sg_all[m]
        dpos = perms[cc][alldst[m] - lo]
        blk = dpos >> 7
        dloc = dpos & 127
        ch = sgl >> 15
        key = (ch * NB + blk)
        order = np.argsort(key, kind="stable")
        sgl, dloc, key = sgl[order], dloc[order], key[order]
        cnt = np.bincount(key, minlength=NB * NCH).reshape(NCH, NB)
        counts_all[cc] = cnt.T
        per_core.append((sgl, dloc, cnt))

    tbc = np.maximum(
        np.ceil(counts_all.max(axis=0) / 128.0).astype(np.int64), 0)
    tbc[:, 0] = np.maximum(tbc[:, 0], 1)   # first-chunk episode inits acc
    T = int(tbc.sum())
    C16 = T * 8

    # stream slot offsets per (c, b)
    cell_off = np.zeros((NCH, NB), np.int64)
    off = 0
    for c in range(NCH):
        for b in range(NB):
            cell_off[c, b] = off
            off += int(tbc[b][c])
    assert off == T

    w1f = np.asarray(W1, np.float32)
    cb16_common = np.concatenate([
        w1f[:P].astype(bf),
        np.asarray(W2, np.float32).astype(bf),
        np.asarray(Wc, np.float32).astype(bf),
        np.broadcast_to(np.arange(P, dtype=np.float32), (P, P)).astype(bf),
        np.eye(P, dtype=np.float32).astype(bf),
    ], axis=1)
    cf32_common = np.concatenate([
        np.broadcast_to(np.asarray(b1, np.float32), (P, HID)),
        np.broadcast_to(np.asarray(b2, np.float32), (P, HID)),
        np.broadcast_to(np.asarray(bc, np.float32), (P, 2)),
    ], axis=1).astype(np.float32)

    in_maps = []
    for cc in range(N_CORES):
        lo = cc * SHARD
        sgl, dloc, cnt = per_core[cc]
        idx16 = np.zeros(T * 128, np.int16)
        dkey = np.full((T * 128,), -1.0, np.float32)
        pos = 0
        for c in range(NCH):
            for b in range(NB):
                n = int(cnt[c, b])
                s0 = int(cell_off[c, b]) * 128
                idx16[s0:s0 + n] = (sgl[pos:pos + n] - c * CHUNK)\
                    .astype(np.int16)
                dkey[s0:s0 + n] = dloc[pos:pos + n]
                pos += n
        assert pos == len(sgl)
        wrapped = np.ascontiguousarray(idx16.reshape(-1, 16).T)  # [16, T*8]
        wrapped = np.tile(wrapped, (8, 1))                       # [128, T*8]
        dst_u = np.ascontiguousarray(dkey.reshape(T, 128).T)     # [128, T]

        xts = np.zeros((IN_F, SHARD_PAD), np.float32)
        xts[:, perms[cc]] = x[lo:lo + SHARD].T
        dv = np.ones(SHARD_PAD, np.float32)
        dv[perms[cc]] = dinv[lo:lo + SHARD]
        f32_part = np.ascontiguousarray(np.concatenate(
            [cf32_common, dv.reshape(NB, P).T], axis=1).astype(np.float32))
        blob = np.concatenate(
            [xts[:P].astype(bf), cb16_common, f32_part.view(bf),
             dst_u.astype(bf), wrapped.view(bf)], axis=1)
        if blob.shape[1] % 2:
            blob = np.concatenate(
                [blob, np.zeros((P, 1), blob.dtype)], axis=1)
        m = {
            "cb16": blob,
            "cb37": np.concatenate(
                [xts[P:].astype(bf), w1f[P:].astype(bf)], axis=1),
        }
        in_maps.append(m)
    return tbc, in_maps, perms


class _Runner:
    """Compile the Bass SPMD program once and execute it on cores 0-7 via
    the PJRT path (modeled on concourse.bass2jax.run_bass_via_pjrt)."""

    def __init__(self, nc, n_cores=8):
        import jax
        import concourse.mybir as mybir
        from jax.sharding import Mesh, PartitionSpec
        from jax.experimental.shard_map import shard_map
        from concourse.bass2jax import (_bass_exec_p, partition_id_tensor,
                                        install_neuronx_cc_hook)
        install_neuronx_cc_hook()
        self.jax = jax
        self.n_cores = n_cores
        in_names, out_names, out_avals = [], [], []
        partition_name = (nc.partition_id_tensor.name
                          if nc.partition_id_tensor else None)
        for alloc in nc.m.functions[0].allocations:
            if not isinstance(alloc, mybir.MemoryLocationSet):
                continue
            name = alloc.memorylocations[0].name
            if alloc.kind == "ExternalInput":
                if name != partition_name:
                    in_names.append(name)
            elif alloc.kind == "ExternalOutput":
                out_names.append(name)
                out_avals.append(jax.core.ShapedArray(
                    tuple(alloc.tensor_shape), mybir.dt.np(alloc.dtype)))
        self.in_names, self.out_names, self.out_avals = \
            in_names, out_names, out_avals
        n_params = len(in_names)
        all_in = list(in_names) + list(out_names)
        if partition_name is not None:
            all_in.append(partition_name)

        def _body(*args):
            operands = list(args)
            if partition_name is not None:
                operands.append(partition_id_tensor())
            outs = _bass_exec_p.bind(
                *operands, out_avals=tuple(out_avals), in_names=tuple(all_in),
                out_names=tuple(out_names), lowering_input_output_aliases=(),
                sim_require_finite=True, sim_require_nnan=True, nc=nc)
            return tuple(outs)

        devices = jax.devices()[:n_cores]
        self.mesh = Mesh(np.asarray(devices), ("core",))
        n_outs = len(out_avals)
        in_specs = (PartitionSpec("core"),) * (n_params + n_outs)
        out_specs = (PartitionSpec("core"),) * n_outs
        self.fn = jax.jit(
            shard_map(_body, mesh=self.mesh, in_specs=in_specs,
                      out_specs=out_specs, check_rep=False),
            keep_unused=True)

    def prep_inputs(self, in_maps):
        import jax
        from jax.sharding import NamedSharding, PartitionSpec
        concat = [np.concatenate([np.asarray(m[name]) for m in in_maps], axis=0)
                  for name in self.in_names]
        zeros = [np.zeros((self.n_cores * a.shape[0], *a.shape[1:]), a.dtype)
                 for a in self.out_avals]
        sharding = NamedSharding(self.mesh, PartitionSpec("core"))
        return [jax.device_put(a, sharding) for a in concat + zeros]

    def run(self, dev_args):
        outs = self.fn(*dev_args)
        self.jax.block_until_ready(outs)
        return outs

    def results(self, outs):
        res = []
        for c in range(self.n_cores):
            d = {}
            for i, name in enumerate(self.out_names):
                d[name] = np.asarray(outs[i]).reshape(
                    self.n_cores, *self.out_avals[i].shape)[c]
            res.append(d)
        return res


_CACHED = {}


def kernel(x, edge_index, W1, b1, W2, b2, Wc, bc):
    tbc, in_maps, perms = _prepare(x, edge_index, W1, b1, W2, b2, Wc, bc)
    key = tbc.tobytes()
    if key not in _CACHED:
        nc = _build([[int(v) for v in row] for row in tbc])
        _CACHED[key] = _Runner(nc)
    r = _CACHED[key]
    dev = r.prep_inputs(in_maps)
    outs = r.run(dev)
    res = r.results(outs)
    full = np.concatenate([res[c]["out"][perms[c]] for c in range(N_CORES)],
                          axis=0)
    return full.astype(np.float32)

